# revision 23
# baseline (speedup 1.0000x reference)
"""GroupLinear Trainium2 kernel.

out[b, g, o] = sum_i x[b, i] * W[g, o, i] + b[g, o]
  x: (4096, 1024) f32, W: (16, 1024, 1024) f32, b: (16, 1024) f32
  out: (4096, 16, 1024) f32

Sharding: groups across the 8 cores (2 groups/core), x replicated.

Layout strategy: x and W are transposed + cast to bf16 on the host so the
contraction dim (i) lands on SBUF partitions with no on-device transposes.
The device kernel is then a pure back-to-back bf16 matmul stream (keeps the
PE p-state ramped to max clock), bias fused into the PSUM->SBUF evacuation,
bf16 output upcast on the host.

Scheduling: the first batch tiles run as pair-interleaved accumulation
chains (2 tiles x 4 chunks = 8 psum banks live) so the PE has ~2x work per
arriving W k-slab and never starves while W streams in; the bias broadcast
rides the output queue to keep the input queue dedicated to x/W.
"""

import sys
import types

sys.path.insert(0, "/opt/trn_rl_repo")

# Provide antenv.axon_hooks (NTFF profile hook registry) if the installed
# antenv lacks it — the axon boot registers its profiling hook here, and
# concourse.bass_utils reads it back when trace=True. Must exist before the
# first jax/axon backend init.
try:
    from antenv import axon_hooks as _axon_hooks  # noqa: F401
except ImportError:
    _m = types.ModuleType("antenv.axon_hooks")
    _m._hook = None

    def _set_hook(hook, _m=_m):
        _m._hook = hook

    def _get_hook(_m=_m):
        return _m._hook

    _m.set_axon_ntff_profile_hook = _set_hook
    _m.get_axon_ntff_profile_hook = _get_hook
    sys.modules["antenv.axon_hooks"] = _m
    try:
        import antenv

        antenv.axon_hooks = _m
    except ImportError:
        pass

from contextlib import ExitStack

import ml_dtypes
import numpy as np

import concourse.bass as bass
import concourse.mybir as mybir
import concourse.tile as tile
from concourse import bacc
from concourse.bass_utils import run_bass_kernel_spmd

F32 = mybir.dt.float32
BF16 = mybir.dt.bfloat16
BF16NP = ml_dtypes.bfloat16

BATCH, D_IN, D_OUT, GROUPS, NCORES = 4096, 1024, 1024, 16, 8
GPC = GROUPS // NCORES  # groups per core
PAIR_TILES = 4          # leading batch tiles run as pair-interleaved chains


def build_nc(batch=BATCH, d_in=D_IN, d_out=D_OUT, gpc=GPC):
    P = 128
    KT = d_in // P           # k-tiles along contraction
    MT = batch // P          # batch tiles
    CW = 512                 # matmul moving free dim (1 psum bank fp32)
    NCH = gpc * d_out // CW  # output chunks per batch tile
    BQ = 512                 # batch columns per x-load chunk

    nc = bacc.Bacc("TRN2", target_bir_lowering=False, debug=False)
    # host-pretransposed: xT[kt, p, b] = x[b, kt*128+p]
    xT = nc.dram_tensor("xT", [KT, P, batch], BF16, kind="ExternalInput").ap()
    # host-pretransposed: WT[g, kt, p, o] = W[g, o, kt*128+p]
    WT = nc.dram_tensor("WT", [gpc, KT, P, d_out], BF16, kind="ExternalInput").ap()
    b = nc.dram_tensor("b", [gpc, d_out], F32, kind="ExternalInput").ap()
    out = nc.dram_tensor("out", [batch, gpc * d_out], BF16, kind="ExternalOutput").ap()

    with ExitStack() as ctx:
        tc = ctx.enter_context(tile.TileContext(nc))
        singles = ctx.enter_context(tc.tile_pool(name="singles", bufs=1))
        out_pool = ctx.enter_context(tc.tile_pool(name="outp", bufs=8))
        ps_mm = ctx.enter_context(tc.tile_pool(name="ps_mm", bufs=8, space="PSUM"))

        # bias broadcast to all 128 partitions, on the output queue so the
        # input queue stays dedicated to the critical x/W stream
        bias_sb = singles.tile([P, gpc * d_out], F32)
        b_bcast = bass.AP(
            tensor=b.tensor, offset=b.offset, ap=[[0, P], [1, gpc * d_out]]
        )
        nc.scalar.dma_start(out=bias_sb[:, :], in_=b_bcast)

        wt = singles.tile([P, KT, gpc * d_out], BF16)
        xt = singles.tile([P, KT, batch], BF16)

        # Input DMA order, one queue (a second input queue just contends for
        # HBM and delays the critical slabs). The first slabs are kept small
        # — x columns for the first tile pair + W group 0 only — because the
        # warmup pair's first pass touches only chunks 0/1; W group 1 and
        # the rest of x follow.
        for kt in range(KT):
            nc.sync.dma_start(out=xt[:, kt, 0 : 2 * P], in_=xT[kt, :, 0 : 2 * P])
            nc.sync.dma_start(out=wt[:, kt, 0:CW], in_=WT[0, kt, :, 0:CW])
            nc.sync.dma_start(out=xt[:, kt, 2 * P : BQ], in_=xT[kt, :, 2 * P : BQ])
            nc.sync.dma_start(out=wt[:, kt, CW:d_out], in_=WT[0, kt, :, CW:d_out])
        for kt in range(KT):
            nc.sync.dma_start(out=wt[:, kt, d_out : 2 * d_out], in_=WT[1, kt])
        # steady-state x loads use 1024-col chunks (2KB partition lines
        # sustain full DMA rate; 1KB lines top out ~200 GB/s)
        col = BQ
        while col < batch:
            w = min(2 * BQ, batch - col)
            for kt in range(KT):
                nc.sync.dma_start(
                    out=xt[:, kt, col : col + w], in_=xT[kt, :, col : col + w]
                )
            col += w

        # NOTE: no PE prewarm. The DVFS controller only steps the PE clock
        # up at stall boundaries; a saturated dummy-matmul stream parks the
        # clock one step below max (2.0 instead of 2.4 GHz) for the whole
        # run. The natural pattern — ~11 matmuls at the 1.2 GHz p-state,
        # one short data-wait, then max clock — is faster.

        def alloc_banks(m):
            return [
                ps_mm.tile([P, CW], F32, tag="ps_mm", name=f"ps_mm_{m}_{c}")
                for c in range(NCH)
            ]

        def chain_step(pss, m, kt):
            lhsT = xt[:, kt, m * P : (m + 1) * P]
            for c in range(NCH):
                nc.tensor.matmul(
                    pss[c][:, :],
                    lhsT,
                    wt[:, kt, c * CW : (c + 1) * CW],
                    start=(kt == 0),
                    stop=(kt == KT - 1),
                )

        def evac_chunk(ps, m, c):
            # bias add + bf16 cast on DVE (GpSimd cannot read PSUM), then
            # the chunk's output DMA
            o_sb = out_pool.tile([P, CW], BF16, tag="outp")
            nc.vector.tensor_add(
                out=o_sb[:, :],
                in0=ps[:, :],
                in1=bias_sb[:, c * CW : (c + 1) * CW],
            )
            nc.scalar.dma_start(
                out=out[m * P : (m + 1) * P, c * CW : (c + 1) * CW],
                in_=o_sb[:, :],
            )

        def evac(pss, m):
            for c in range(NCH):
                evac_chunk(pss[c], m, c)

        # Warmup: two quad-interleaved phases over the first 4 batch tiles.
        # Phase A touches only W-group-0 chunks (0/1) — 4 tiles x 2 chunks =
        # 8 psum banks, ~14us of PE runway against ~9us of critical DMA
        # (x q0 + W g0). Phase B (chunks 2/3) runs while nothing is left to
        # wait for: W g1 finished streaming during phase A.
        for clo in (0, NCH // 2):
            pss = {
                m: {
                    c: ps_mm.tile([P, CW], F32, tag="ps_mm", name=f"ps_{m}_{c}")
                    for c in (clo, clo + 1)
                }
                for m in range(4)
            }
            for kt in range(KT):
                for c in (clo, clo + 1):
                    for m in range(4):
                        nc.tensor.matmul(
                            pss[m][c][:, :],
                            xt[:, kt, m * P : (m + 1) * P],
                            wt[:, kt, c * CW : (c + 1) * CW],
                            start=(kt == 0),
                            stop=(kt == KT - 1),
                        )
            for m in range(4):
                for c in (clo, clo + 1):
                    evac_chunk(pss[m][c], m, c)

        # steady state: per-tile chains (4 banks, short evac tail)
        for m in range(4, MT - 1):
            pss = alloc_banks(m)
            for kt in range(KT):
                chain_step(pss, m, kt)
            evac(pss, m)

        # last tile runs chunk-major so each chunk's evacuation overlaps the
        # remaining chunks' matmuls; only the final chunk's evac trails
        m = MT - 1
        pss = alloc_banks(m)
        for c in range(NCH):
            for kt in range(KT):
                nc.tensor.matmul(
                    pss[c][:, :],
                    xt[:, kt, m * P : (m + 1) * P],
                    wt[:, kt, c * CW : (c + 1) * CW],
                    start=(kt == 0),
                    stop=(kt == KT - 1),
                )
            evac_chunk(pss[c], m, c)

    nc.finalize()
    return nc


_NC_CACHE = {}


def _get_nc(key=(BATCH, D_IN, D_OUT, GPC)):
    if key not in _NC_CACHE:
        _NC_CACHE[key] = build_nc(*key)
    return _NC_CACHE[key]


def _run(inputs, trace=False):
    x = np.asarray(inputs["x"], dtype=np.float32)
    W = np.asarray(inputs["W"], dtype=np.float32)
    b = np.asarray(inputs["b"], dtype=np.float32)

    KT = D_IN // 128
    # xT[kt, p, b] = x[b, kt*128+p]
    xT = np.ascontiguousarray(x.astype(BF16NP).T).reshape(KT, 128, BATCH)
    W_bf = W.astype(BF16NP)

    nc = _get_nc()
    in_maps = []
    for c in range(NCORES):
        # WT[g, kt, p, o] = W[c*GPC+g, o, kt*128+p]
        Wc = np.ascontiguousarray(
            W_bf[c * GPC : (c + 1) * GPC].transpose(0, 2, 1)
        ).reshape(GPC, KT, 128, D_OUT)
        in_maps.append(
            {
                "xT": xT,
                "WT": Wc,
                "b": np.ascontiguousarray(b[c * GPC : (c + 1) * GPC]),
            }
        )
    res = run_bass_kernel_spmd(nc, in_maps, core_ids=list(range(NCORES)), trace=trace)
    shards = [r["out"] for r in res.results]
    full = np.concatenate(shards, axis=1).astype(np.float32)
    return full.reshape(BATCH, GROUPS, D_OUT), res


def kernel(**inputs):
    out, _ = _run(inputs, trace=False)
    return out


# revision 28
# speedup vs baseline: 1.0231x; 1.0231x over previous
"""GroupLinear Trainium2 kernel.

out[b, g, o] = sum_i x[b, i] * W[g, o, i] + b[g, o]
  x: (4096, 1024) f32, W: (16, 1024, 1024) f32, b: (16, 1024) f32
  out: (4096, 16, 1024) f32

Sharding: groups across the 8 cores (2 groups/core), x replicated.

Layout strategy: x and W are transposed + cast to bf16 on the host so the
contraction dim (i) lands on SBUF partitions with no on-device transposes.
The device kernel is then a pure back-to-back bf16 matmul stream (keeps the
PE p-state ramped to max clock), bias fused into the PSUM->SBUF evacuation,
bf16 output upcast on the host.

Scheduling: the first 4 batch tiles run as two quad-interleaved phases
(4 tiles x 2 chunks = 8 psum banks live, W group 0 then group 1) so the PE
has more work per arriving W k-slab than the DMA can deliver and never
starves while W streams in; the bias broadcast rides the output queue to
keep the input queue dedicated to x/W; the last tile runs chunk-major so
its evacuation overlaps its own matmuls.
"""

import sys
import types

sys.path.insert(0, "/opt/trn_rl_repo")

# Provide antenv.axon_hooks (NTFF profile hook registry) if the installed
# antenv lacks it — the axon boot registers its profiling hook here, and
# concourse.bass_utils reads it back when trace=True. Must exist before the
# first jax/axon backend init.
try:
    from antenv import axon_hooks as _axon_hooks  # noqa: F401
except ImportError:
    _m = types.ModuleType("antenv.axon_hooks")
    _m._hook = None

    def _set_hook(hook, _m=_m):
        _m._hook = hook

    def _get_hook(_m=_m):
        return _m._hook

    _m.set_axon_ntff_profile_hook = _set_hook
    _m.get_axon_ntff_profile_hook = _get_hook
    sys.modules["antenv.axon_hooks"] = _m
    try:
        import antenv

        antenv.axon_hooks = _m
    except ImportError:
        pass

from contextlib import ExitStack

import ml_dtypes
import numpy as np

import concourse.bass as bass
import concourse.mybir as mybir
import concourse.tile as tile
from concourse import bacc
from concourse.bass_utils import run_bass_kernel_spmd

F32 = mybir.dt.float32
BF16 = mybir.dt.bfloat16
BF16NP = ml_dtypes.bfloat16

BATCH, D_IN, D_OUT, GROUPS, NCORES = 4096, 1024, 1024, 16, 8
GPC = GROUPS // NCORES  # groups per core


def build_nc(batch=BATCH, d_in=D_IN, d_out=D_OUT, gpc=GPC):
    P = 128
    KT = d_in // P           # k-tiles along contraction
    MT = batch // P          # batch tiles
    CW = 512                 # matmul moving free dim (1 psum bank fp32)
    NCH = gpc * d_out // CW  # output chunks per batch tile
    BQ = 512                 # batch columns per x-load chunk

    nc = bacc.Bacc("TRN2", target_bir_lowering=False, debug=False)
    # host-pretransposed: xT[kt, p, b] = x[b, kt*128+p]
    xT = nc.dram_tensor("xT", [KT, P, batch], BF16, kind="ExternalInput").ap()
    # host-pretransposed: WT[g, kt, p, o] = W[g, o, kt*128+p]
    WT = nc.dram_tensor("WT", [gpc, KT, P, d_out], BF16, kind="ExternalInput").ap()
    b = nc.dram_tensor("b", [gpc, d_out], F32, kind="ExternalInput").ap()
    out = nc.dram_tensor("out", [batch, gpc * d_out], BF16, kind="ExternalOutput").ap()

    with ExitStack() as ctx:
        tc = ctx.enter_context(tile.TileContext(nc))
        singles = ctx.enter_context(tc.tile_pool(name="singles", bufs=1))
        out_pool = ctx.enter_context(tc.tile_pool(name="outp", bufs=8))
        ps_mm = ctx.enter_context(tc.tile_pool(name="ps_mm", bufs=8, space="PSUM"))

        # bias broadcast to all 128 partitions, on the output queue so the
        # input queue stays dedicated to the critical x/W stream
        bias_sb = singles.tile([P, gpc * d_out], F32)
        b_bcast = bass.AP(
            tensor=b.tensor, offset=b.offset, ap=[[0, P], [1, gpc * d_out]]
        )
        nc.scalar.dma_start(out=bias_sb[:, :], in_=b_bcast)

        wt = singles.tile([P, KT, gpc * d_out], BF16)
        xt = singles.tile([P, KT, batch], BF16)

        # Input DMA order, one queue (a second input queue just contends for
        # HBM and delays the critical slabs): x columns for the warmup tiles
        # + W group 0 first (warmup phase A touches only chunks 0/1), then
        # W group 1, then the rest of x.
        for kt in range(KT):
            nc.sync.dma_start(out=xt[:, kt, 0:BQ], in_=xT[kt, :, 0:BQ])
            nc.sync.dma_start(out=wt[:, kt, 0:d_out], in_=WT[0, kt])
        for kt in range(KT):
            nc.sync.dma_start(out=wt[:, kt, d_out : 2 * d_out], in_=WT[1, kt])
        # steady-state x loads use 1024-col chunks (2KB partition lines
        # sustain full DMA rate; 1KB lines top out ~200 GB/s)
        col = BQ
        while col < batch:
            w = min(2 * BQ, batch - col)
            for kt in range(KT):
                nc.sync.dma_start(
                    out=xt[:, kt, col : col + w], in_=xT[kt, :, col : col + w]
                )
            col += w

        # NOTE: no PE prewarm. The DVFS controller only steps the PE clock
        # up at stall boundaries; a saturated dummy-matmul stream parks the
        # clock one step below max (2.0 instead of 2.4 GHz) for the whole
        # run. The natural pattern — ~11 matmuls at the 1.2 GHz p-state,
        # one short data-wait, then max clock — is faster.

        def alloc_banks(m):
            return [
                ps_mm.tile([P, CW], F32, tag="ps_mm", name=f"ps_mm_{m}_{c}")
                for c in range(NCH)
            ]

        def chain_step(pss, m, kt):
            lhsT = xt[:, kt, m * P : (m + 1) * P]
            for c in range(NCH):
                nc.tensor.matmul(
                    pss[c][:, :],
                    lhsT,
                    wt[:, kt, c * CW : (c + 1) * CW],
                    start=(kt == 0),
                    stop=(kt == KT - 1),
                )

        def evac_chunk(ps, m, c):
            # bias add + bf16 cast on DVE (GpSimd cannot read PSUM), then
            # the chunk's output DMA
            o_sb = out_pool.tile([P, CW], BF16, tag="outp")
            nc.vector.tensor_add(
                out=o_sb[:, :],
                in0=ps[:, :],
                in1=bias_sb[:, c * CW : (c + 1) * CW],
            )
            nc.scalar.dma_start(
                out=out[m * P : (m + 1) * P, c * CW : (c + 1) * CW],
                in_=o_sb[:, :],
            )

        def evac(pss, m):
            for c in range(NCH):
                evac_chunk(pss[c], m, c)

        # Warmup: two quad-interleaved phases over the first 4 batch tiles.
        # Phase A touches only W-group-0 chunks (0/1) — 4 tiles x 2 chunks =
        # 8 psum banks, ~14us of PE runway against ~9us of critical DMA
        # (x q0 + W g0). Phase B (chunks 2/3) runs while nothing is left to
        # wait for: W g1 finished streaming during phase A.
        for clo in (0, NCH // 2):
            pss = {
                m: {
                    c: ps_mm.tile([P, CW], F32, tag="ps_mm", name=f"ps_{m}_{c}")
                    for c in (clo, clo + 1)
                }
                for m in range(4)
            }
            for kt in range(KT):
                for m in range(4):
                    lhsT = xt[:, kt, m * P : (m + 1) * P]
                    for c in (clo, clo + 1):
                        nc.tensor.matmul(
                            pss[m][c][:, :],
                            lhsT,
                            wt[:, kt, c * CW : (c + 1) * CW],
                            start=(kt == 0),
                            stop=(kt == KT - 1),
                        )
            for m in range(4):
                for c in (clo, clo + 1):
                    evac_chunk(pss[m][c], m, c)

        # steady state: per-tile chains (4 banks, short evac tail)
        for m in range(4, MT - 1):
            pss = alloc_banks(m)
            for kt in range(KT):
                chain_step(pss, m, kt)
            evac(pss, m)

        # last tile runs chunk-major so each chunk's evacuation overlaps the
        # remaining chunks' matmuls; only the final chunk's evac trails
        m = MT - 1
        pss = alloc_banks(m)
        for c in range(NCH):
            for kt in range(KT):
                nc.tensor.matmul(
                    pss[c][:, :],
                    xt[:, kt, m * P : (m + 1) * P],
                    wt[:, kt, c * CW : (c + 1) * CW],
                    start=(kt == 0),
                    stop=(kt == KT - 1),
                )
            evac_chunk(pss[c], m, c)

    nc.finalize()
    return nc


_NC_CACHE = {}


def _get_nc(key=(BATCH, D_IN, D_OUT, GPC)):
    if key not in _NC_CACHE:
        _NC_CACHE[key] = build_nc(*key)
    return _NC_CACHE[key]


def _run(inputs, trace=False):
    x = np.asarray(inputs["x"], dtype=np.float32)
    W = np.asarray(inputs["W"], dtype=np.float32)
    b = np.asarray(inputs["b"], dtype=np.float32)

    KT = D_IN // 128
    # xT[kt, p, b] = x[b, kt*128+p]
    xT = np.ascontiguousarray(x.astype(BF16NP).T).reshape(KT, 128, BATCH)
    W_bf = W.astype(BF16NP)

    nc = _get_nc()
    in_maps = []
    for c in range(NCORES):
        # WT[g, kt, p, o] = W[c*GPC+g, o, kt*128+p]
        Wc = np.ascontiguousarray(
            W_bf[c * GPC : (c + 1) * GPC].transpose(0, 2, 1)
        ).reshape(GPC, KT, 128, D_OUT)
        in_maps.append(
            {
                "xT": xT,
                "WT": Wc,
                "b": np.ascontiguousarray(b[c * GPC : (c + 1) * GPC]),
            }
        )
    res = run_bass_kernel_spmd(nc, in_maps, core_ids=list(range(NCORES)), trace=trace)
    shards = [r["out"] for r in res.results]
    full = np.concatenate(shards, axis=1).astype(np.float32)
    return full.reshape(BATCH, GROUPS, D_OUT), res


def kernel(**inputs):
    out, _ = _run(inputs, trace=False)
    return out
